# revision 2
# baseline (speedup 1.0000x reference)
"""GraphConv VAE encoder (3x GraphConv + reparameterization) on 8 Trainium2 cores.

Strategy (graph/data parallel, dst-sharded), v2:
  - Nodes padded to NPAD = 8*SH and sharded by dst across 8 cores.
  - Layer-1 projection hp = (feat @ W1) * ns computed on each core for its own
    node shard (host pre-transposes feat), then AllGather -> full bf16 table.
  - Edges are dst-sorted into (supergroup, src-bucket) cells; a supergroup is
    SG=16 sblocks of 128 dst nodes whose PSUM accumulators live concurrently.
    Within a cell, edges sort by sblock; 128-edge chunks may span adjacent
    sblocks, handled by one submatmul per (chunk, sblock) with an S column
    that blanks foreign edges (dstloc sentinel).  Cell-level padding only:
    ~3% pad vs ~25% for per-(sblock,bucket) cells.
  - Per chunk: dma_gather the source rows (partition = edge slot), build
    one-hot S via iota==dstloc on DVE, matmul into per-sblock PSUM.
  - Layer 1 sink: h = relu(agg*nd + b1); hs = h*ns stored row-major (no
    transpose), AllGather -> full hs table.
  - Layers 2/3 exploit linearity of segment_sum: aggregate hs[src] FIRST
    (transposed: aggT = rows^T @ S), then project once per sblock:
    out = (aggT^T @ [W_mu|W_ls]) * nd + b, fused with the VAE epilogue
    mu + noise * exp(log_sigma).  No replicated projection pass, one fewer
    AllGather, and pass-2 gather rows stay 256B.
"""

import sys

sys.path.insert(0, '/opt/trn_rl_repo')

import numpy as np
import ml_dtypes

import concourse.bass as bass
import concourse.bacc as bacc
import concourse.mybir as mybir
import concourse.tile as tile
from concourse import library_config
from concourse.tile_rust import add_dep_helper
from concourse.vector_clock import ScopedClock
from concourse.bass_utils import run_bass_kernel_spmd

BF16 = mybir.dt.bfloat16
F32 = mybir.dt.float32
NPBF16 = ml_dtypes.bfloat16

NC = 8          # cores
P = 128         # partitions / sblock width
SG = 8          # sblocks per supergroup (one PSUM bank each; a PSUM
                # bank admits only one pending accumulation group)
NBUCK = 4       # src-range buckets (int16 gather index limit)
GRP = 16        # submatmuls per S-build batch
PAD_DSTLOC = 300.0  # dstloc sentinel for pad/foreign slots (never matches iota)


def _patch_tile_drain():
    """This walrus build rejects >1 sync-wait on the kernel-tail Drain; spread
    the waits across chained drains."""
    if getattr(tile.TileContext, "_drain_patched", False):
        return

    def patched(self, tick_clock, wait_clock):
        drain_inst = self.nc.sync.drain()
        wait_clock.add_sem_waits(drain_inst.ins,
                                 ScopedClock({None: tick_clock.global_clock}))
        si = drain_inst.ins.sync_info
        if si is not None and si.on_wait and len(si.on_wait) > 1:
            waits = list(si.on_wait)
            si.on_wait = waits[:1]
            for w in waits[1:]:
                d2 = self.nc.sync.drain()
                d2.ins.sync_info = mybir.SyncInfo(on_wait=[w], on_update=[])
        self.nc.all_engine_barrier()
        assert self.sems is not None
        popped = self.nc._tile_sem_poison_stack.pop()
        assert popped is self._sem_poison
        self.nc.clear_and_free_semaphores(list(self.sems.allocated().values()))
        self.nc.all_engine_barrier()

    tile.TileContext._drain_and_barrier = patched
    tile.TileContext._drain_patched = True


def _build_template(edges, n_nodes, npad):
    """Host-side edge preprocessing shared by both gather passes.

    Returns the SPMD-shared template and per-core slot data (int16 gather
    indices wrapped for dma_gather, dstloc column per submatmul).
    """
    src = edges[0].astype(np.int64)
    dst = edges[1].astype(np.int64)
    sh = npad // NC          # nodes per core shard
    nsb = sh // P            # sblocks per core
    brows = npad // NBUCK    # rows per gather bucket
    n_sg = (nsb + SG - 1) // SG

    core = dst // sh
    k = (dst % sh) // P            # absolute sblock
    g = k // SG
    b = src // brows
    dloc = dst % P

    # per-core cell id, (g major, b minor)
    cell = (core * n_sg + g) * NBUCK + b
    n_cells_pc = n_sg * NBUCK
    cnt = np.bincount(cell, minlength=NC * n_cells_pc).reshape(
        NC, n_cells_pc)
    C = np.maximum(1, -(-cnt.max(axis=0) // P))      # chunks per cell
    cell_slots = C * P
    cell_off = np.concatenate([[0], np.cumsum(cell_slots)[:-1]])
    total_slots = int(cell_slots.sum())
    n_chunks = total_slots // P

    # per-core slot assignment: sort by (cell, k) so chunks walk sblocks
    order = np.lexsort((k, cell))
    cell_sorted = cell[order]
    cell_start = np.searchsorted(cell_sorted, np.arange(NC * n_cells_pc),
                                 side='left')
    rank = np.arange(len(order)) - cell_start[cell_sorted]
    slot = cell_off[cell_sorted % n_cells_pc] + rank

    idx_vals = np.zeros((NC, total_slots), np.int16)
    k_slot = np.full((NC, total_slots), -1, np.int64)
    dl_slot = np.zeros((NC, total_slots), np.int64)
    csrc = (src - b * brows)[order]
    ccore = core[order]
    idx_vals[ccore, slot] = csrc.astype(np.int16)
    k_slot[ccore, slot] = k[order]
    dl_slot[ccore, slot] = dloc[order]

    # calls: one dma_gather per cell; submatmuls: per (chunk, union sblock)
    calls = []               # (cell_i, b, slot_off, num_idxs)
    subs = []                # (cell_i, local_chunk_j, k_abs, start, stop)
    have_k = set()
    for ci in range(n_cells_pc):
        gg, bb = divmod(ci, NBUCK)
        off = int(cell_off[ci])
        ni = int(cell_slots[ci])
        calls.append((ci, bb, off, ni))
        kc = k_slot[:, off:off + ni].reshape(NC, ni // P, P)
        for j in range(ni // P):
            ks = np.unique(kc[:, j, :])
            ks = ks[ks >= 0]
            if len(ks) == 0:
                ks = [gg * SG]   # pure-pad chunk: park on first sblock of sg
            for kk in ks:
                have_k.add(int(kk))
                subs.append([ci, j, int(kk), False, False])
    for kk in range(nsb):
        if kk not in have_k:
            # sblock with no edges on any core: attach an all-sentinel
            # submatmul to its supergroup's first cell/chunk so the PSUM
            # accumulator gets a start+stop (writes zeros).
            ci = (kk // SG) * NBUCK
            pos = next(i for i, s in enumerate(subs) if s[0] == ci)
            subs.insert(pos, [ci, 0, kk, False, False])
    # start/stop = first/last submatmul per sblock
    seen_first = set()
    for s in subs:
        if s[2] not in seen_first:
            s[3] = True
            seen_first.add(s[2])
    last_of = {}
    for i, s in enumerate(subs):
        last_of[s[2]] = i
    for i in last_of.values():
        subs[i][4] = True

    n_sub = len(subs)

    # dl table: one [128] column per submatmul
    dl = np.full((NC, n_sub, P), PAD_DSTLOC, np.float32)
    for si, (ci, j, kk, st, sp) in enumerate(subs):
        off = int(cell_off[ci]) + j * P
        sel = k_slot[:, off:off + P] == kk
        col = np.where(sel, dl_slot[:, off:off + P], PAD_DSTLOC)
        dl[:, si, :] = col
    dl = dl.transpose(0, 2, 1).astype(NPBF16)        # [NC, 128, n_sub]
    dl = np.ascontiguousarray(dl)

    # wrap indices per call: within a call, slot s -> [s%16, off//16 + s//16]
    ni16 = total_slots // 16
    idx16 = np.zeros((NC, 16, ni16), np.int16)
    for (_, _, off, ni) in calls:
        blk = idx_vals[:, off:off + ni].reshape(NC, ni // 16, 16)
        idx16[:, :, off // 16:(off + ni) // 16] = blk.transpose(0, 2, 1)
    idx16 = np.tile(idx16, (1, 8, 1))  # replicate to 128 partitions

    tpl = dict(sh=sh, nsb=nsb, brows=brows, n_sg=n_sg, calls=calls,
               subs=subs, C=C, cell_off=cell_off, n_chunks=n_chunks,
               total_slots=total_slots, ni16=ni16, n_sub=n_sub)
    return tpl, idx16, dl


def _build(feat, edges, W1, b1, W_mu, b_mu, W_ls, b_ls, noise):
    import os
    skip = os.environ.get("K_SKIP", "")
    repeat = int(os.environ.get("K_REPEAT", "1"))
    split = os.environ.get("K_SPLITAG", "") == "1"
    # 4 SWDGE queues with each dma_gather split across 2 of them measured
    # fastest on HW (descriptor processing is the gather bottleneck; queue
    # parallelism is the lever -- bytes are nearly free).
    NQ = int(os.environ.get("K_NQ", "4"))
    QSPLIT = int(os.environ.get("K_QSPLIT", "2"))
    elem512 = os.environ.get("K_ELEM512", "") == "1"
    spacket = os.environ.get("K_SPACKET", "") == "1"
    N, IN = feat.shape
    OUT = W1.shape[1]
    F2 = 2 * OUT
    TW = 2 * OUT if elem512 else OUT         # gather-table row width (elems)
    assert OUT == P
    npad = -(-N // (NC * P)) * NC * P        # multiple of 8*128
    while npad % (NBUCK * P) != 0:
        npad += NC * P
    sh = npad // NC
    brows = npad // NBUCK
    assert brows <= 32768
    nsb = sh // P
    kin = IN // P

    hsh = sh // 2
    khalf = hsh // P                 # sblocks per half (49)
    if split:
        # remap src -> table row [all cores half0 | all cores half1] so each
        # AllGather half fills a prefix of the gather windows.
        v = edges[0].astype(np.int64)
        cc = v // sh
        rr = v % sh
        src2 = (rr // hsh) * (npad // 2) + cc * hsh + (rr % hsh)
        edges_t = np.stack([src2, edges[1].astype(np.int64)])
        assert hsh % P == 0 and khalf % 7 == 0
    else:
        edges_t = edges
    tpl, idx16, dl_host = _build_template(edges_t, N, npad)
    calls, subs = tpl['calls'], tpl['subs']
    n_chunks, ni16, n_sub = tpl['n_chunks'], tpl['ni16'], tpl['n_sub']
    C, cell_off, n_sg = tpl['C'], tpl['cell_off'], tpl['n_sg']

    # ---- host-side numeric prep (degrees from the index arrays) ----
    deg_out = np.bincount(edges[0], minlength=npad).astype(np.float64)
    deg_in = np.bincount(edges[1], minlength=npad).astype(np.float64)
    ns = np.clip(deg_out, 1.0, None) ** -0.5
    nd = np.clip(deg_in, 1.0, None) ** -0.5
    ns[N:] = 0.0
    nd[N:] = 0.0
    ns = ns.astype(np.float32)
    nd = nd.astype(np.float32)

    featp = np.zeros((npad, IN), np.float32)
    featp[:N] = feat
    noisep = np.zeros((npad, OUT), np.float32)
    noisep[:N] = noise

    featb = featp.astype(NPBF16)
    W1b = np.ascontiguousarray(W1.astype(NPBF16))
    W23 = np.concatenate([W_mu, W_ls], axis=1)
    W23b = np.ascontiguousarray(W23.astype(NPBF16))
    # W1 as [128, kin, 128]: [p, kc, j] = W1[kc*128+p, j]
    W1sb = np.ascontiguousarray(W1b.reshape(kin, P, OUT).transpose(1, 0, 2))

    # iota2[p, d*GRP + c] = d — packed last dim so the S-build is_equal
    # qualifies for the DVE 2x_1p mode (vs 1x with a stride-0 last dim).
    iota2 = np.repeat(np.arange(P, dtype=np.float32), GRP)[None, :]
    iota2 = iota2.repeat(P, 0).astype(NPBF16)          # [128, 128*GRP]
    b1r = np.tile(b1[None, :].astype(np.float32), (P, 1))
    bmur = np.tile(b_mu[None, :].astype(np.float32), (P, 1))
    blsr = np.tile(b_ls[None, :].astype(np.float32), (P, 1))

    in_maps = []
    for c in range(NC):
        rows = slice(c * sh, (c + 1) * sh)
        fsh = featb[rows]                               # [sh, IN]
        featT = np.ascontiguousarray(
            fsh.T.reshape(kin, P, sh).transpose(1, 0, 2).reshape(P, kin * sh))
        nsc = np.ascontiguousarray(
            ns[rows].reshape(nsb, P).T)                 # [128, nsb]
        ndc = np.ascontiguousarray(nd[rows].reshape(nsb, P).T)
        noc = np.ascontiguousarray(
            noisep[rows].reshape(nsb, P, OUT).transpose(1, 0, 2)
            .reshape(P, nsb * OUT))                     # [128, nsb*128]
        in_maps.append({
            "featT": featT, "W1sb": W1sb.reshape(P, kin * OUT),
            "W23sb": W23b, "b1r": b1r, "bmur": bmur, "blsr": blsr,
            "nsc": nsc, "ndc": ndc, "noise_sb": noc,
            "iota2": iota2,
            "idx16": np.ascontiguousarray(idx16[c]),
            "dstloc": np.ascontiguousarray(dl_host[c]),
        })

    # ---------------- device program ----------------
    _patch_tile_drain()
    nc = bacc.Bacc('TRN2', target_bir_lowering=False, debug=False,
                   num_swdge_queues=NQ)

    featT_d = nc.dram_tensor("featT", [P, kin * sh], BF16, kind="ExternalInput")
    W1_d = nc.dram_tensor("W1sb", [P, kin * OUT], BF16, kind="ExternalInput")
    W23_d = nc.dram_tensor("W23sb", [P, F2], BF16, kind="ExternalInput")
    b1_d = nc.dram_tensor("b1r", [P, OUT], F32, kind="ExternalInput")
    bmu_d = nc.dram_tensor("bmur", [P, OUT], F32, kind="ExternalInput")
    bls_d = nc.dram_tensor("blsr", [P, OUT], F32, kind="ExternalInput")
    ns_d = nc.dram_tensor("nsc", [P, nsb], F32, kind="ExternalInput")
    nd_d = nc.dram_tensor("ndc", [P, nsb], F32, kind="ExternalInput")
    noise_d = nc.dram_tensor("noise_sb", [P, nsb * OUT], F32,
                             kind="ExternalInput")
    iota_d = nc.dram_tensor("iota2", [P, P * GRP], BF16, kind="ExternalInput")
    idx_d = nc.dram_tensor("idx16", [P, ni16], mybir.dt.int16,
                           kind="ExternalInput")
    dl_d = nc.dram_tensor("dstloc", [P, n_sub], BF16, kind="ExternalInput")
    y_d = nc.dram_tensor("y", [sh, OUT], F32, kind="ExternalOutput")

    replica = [list(range(NC))]

    with tile.TileContext(nc) as tc:
        import contextlib
        with contextlib.ExitStack() as ctx:
            dram = ctx.enter_context(tc.tile_pool(name="dram", bufs=1,
                                                  space="DRAM"))
            cpool = ctx.enter_context(tc.tile_pool(name="const", bufs=1))
            # PSUM: 8 banks, one pending accumulation group per bank -> a
            # single pool of 8 bank-sized bufs shared by all matmul sinks.
            psA = ctx.enter_context(tc.tile_pool(name="psA", bufs=8,
                                                 space="PSUM"))

            if split:
                hp_bounce = [dram.tile([hsh, OUT], BF16, tag="hp_bounce_a",
                                       name="hp_bounce_a"),
                             dram.tile([hsh, OUT], BF16, tag="hp_bounce_b",
                                       name="hp_bounce_b")]
                hs_bounce = [dram.tile([hsh, OUT], BF16, tag="hs_bounce_a",
                                       name="hs_bounce_a"),
                             dram.tile([hsh, OUT], BF16, tag="hs_bounce_b",
                                       name="hs_bounce_b")]
                hp_fulls = [[dram.tile([npad // 2, OUT], BF16,
                                       tag=f"hp_full_{r}{h}",
                                       addr_space="Shared",
                                       name=f"hp_full_{r}{h}")
                             for h in "ab"] for r in range(repeat)]
                hs_fulls = [[dram.tile([npad // 2, OUT], BF16,
                                       tag=f"hs_full_{r}{h}",
                                       addr_space="Shared",
                                       name=f"hs_full_{r}{h}")
                             for h in "ab"] for r in range(repeat)]
            else:
                hp_bounce = dram.tile([sh, TW], BF16, tag="hp_bounce")
                hs_bounce = dram.tile([sh, TW], BF16, tag="hs_bounce")
                hp_fulls = [dram.tile([npad, TW], BF16, tag=f"hp_full_{r}",
                                      addr_space="Shared",
                                      name=f"hp_full_{r}")
                            for r in range(repeat)]
                hs_fulls = [dram.tile([npad, TW], BF16, tag=f"hs_full_{r}",
                                      addr_space="Shared",
                                      name=f"hs_full_{r}")
                            for r in range(repeat)]

            # constants
            W1_t = cpool.tile([P, kin, OUT], BF16, tag="w1")
            W23_t = cpool.tile([P, F2], BF16, tag="w23")
            b1_t = cpool.tile([P, OUT], F32, tag="b1")
            bmu_t = cpool.tile([P, OUT], F32, tag="bmu")
            bls_t = cpool.tile([P, OUT], F32, tag="bls")
            ns_t = cpool.tile([P, nsb], F32, tag="ns")
            nd_t = cpool.tile([P, nsb], F32, tag="nd")
            iota_t = cpool.tile([P, P, GRP], BF16, tag="iota")
            idx_t = cpool.tile([P, ni16], mybir.dt.int16, tag="idx")
            dl_t = cpool.tile([P, n_sub], BF16, tag="dl")
            nc.sync.dma_start(out=W1_t[:], in_=W1_d[:].rearrange(
                "p (k o) -> p k o", k=kin))
            nc.sync.dma_start(out=W23_t[:], in_=W23_d[:])
            nc.sync.dma_start(out=b1_t[:], in_=b1_d[:])
            nc.sync.dma_start(out=bmu_t[:], in_=bmu_d[:])
            nc.sync.dma_start(out=bls_t[:], in_=bls_d[:])
            nc.sync.dma_start(out=ns_t[:], in_=ns_d[:])
            nc.sync.dma_start(out=nd_t[:], in_=nd_d[:])
            nc.sync.dma_start(out=iota_t[:], in_=iota_d[:].rearrange(
                "p (a b) -> p a b", a=P))
            nc.sync.dma_start(out=idx_t[:], in_=idx_d[:])
            nc.sync.dma_start(out=dl_t[:], in_=dl_d[:])

            reload_inst = nc.gpsimd.load_library(library_config.mlp)

            max_call_chunks = int(C.max())

            def gather_pass(table_aps, gpool, spool, transpose_acc,
                            chunk_sink):
                """Per-cell dma_gather calls; S build per 8 submatmuls;
                one matmul per (chunk, sblock) into per-sblock psums.
                chunk_sink(k_abs, ps) fires when a supergroup's sblock is
                complete (all its submatmuls done)."""
                ps_of = {}
                s8 = None
                si = 0
                sub_i = 0
                for (ci, bb, off, nidx) in calls:
                    gg = ci // NBUCK
                    if bb == 0:
                        for kk in range(gg * SG, min((gg + 1) * SG, nsb)):
                            ps_of[kk] = psA.tile([P, P], F32, tag="acc",
                                                 name=f"acc_{kk}")[:]
                    gt = gpool.tile([P, max_call_chunks, TW], BF16,
                                    tag="gt")
                    nch = nidx // P
                    bounds = [(nch * s) // QSPLIT for s in range(QSPLIT + 1)]
                    for s in range(QSPLIT):
                        c0, c1 = bounds[s], bounds[s + 1]
                        if c0 == c1:
                            continue
                        o0 = off + c0 * P
                        ni_s = (c1 - c0) * P
                        gi = nc.gpsimd.dma_gather(
                            out_ap=gt[:, c0:c1, :],
                            in_ap=table_aps[bb],
                            idxs_ap=idx_t[:, o0 // 16:(o0 + ni_s) // 16],
                            num_idxs=ni_s, num_idxs_reg=ni_s,
                            elem_size=TW, single_packet=spacket,
                            queue_num=(ci * QSPLIT + s) % NQ)
                        add_dep_helper(gi.ins, reload_inst.ins, sync=False)
                    while sub_i < n_sub and subs[sub_i][0] == ci:
                        if si % GRP == 0:
                            s8 = spool.tile([P, P, GRP], BF16, tag="s8")
                            n8 = min(GRP, n_sub - si)
                            nc.vector.tensor_tensor(
                                out=s8[:, :, :n8],
                                in0=iota_t[:, :, :n8],
                                in1=dl_t[:, None, si:si + n8]
                                .to_broadcast([P, P, n8]),
                                op=mybir.AluOpType.is_equal)
                        (ci_, j, kk, st, sp) = subs[sub_i]
                        if transpose_acc:
                            nc.tensor.matmul(
                                ps_of[kk], lhsT=gt[:, j, :OUT],
                                rhs=s8[:, :, si % GRP], start=st, stop=sp)
                        else:
                            nc.tensor.matmul(
                                ps_of[kk], lhsT=s8[:, :, si % GRP],
                                rhs=gt[:, j, :OUT], start=st, stop=sp)
                        if sp:
                            chunk_sink(kk, ps_of[kk])
                        si += 1
                        sub_i += 1
                assert sub_i == n_sub

            def one_iter(hp_full, hs_full):
                # ------------- P1: hp = (feat @ W1) * ns -------------
                with tc.tile_pool(name="featT", bufs=1) as fpool, \
                     tc.tile_pool(name="p1work", bufs=4) as wpool:
                    fT = fpool.tile([P, kin, sh], BF16, tag="fT", name="fT")
                    STRIP = 7 if split else 8
                    for s0 in range(0, nsb, STRIP):
                        s1 = min(s0 + STRIP, nsb)
                        nc.sync.dma_start(
                            out=fT[:, :, s0 * P:s1 * P],
                            in_=featT_d[:].rearrange(
                                "p (k s) -> p k s", k=kin)[:, :, s0 * P:s1 * P])
                        strip = wpool.tile([P, STRIP, OUT], BF16,
                                           tag="hpstrip", name="hpstrip")
                        for rt in range(s0, s1):
                            ps = psA.tile([P, OUT], F32, tag="acc",
                                          name="p1ps")[:]
                            for kc in range(kin):
                                nc.tensor.matmul(
                                    ps,
                                    lhsT=fT[:, kc, rt * P:(rt + 1) * P],
                                    rhs=W1_t[:, kc, :],
                                    start=(kc == 0), stop=(kc == kin - 1))
                            nc.vector.tensor_scalar_mul(
                                strip[:, rt - s0, :], ps,
                                ns_t[:, rt:rt + 1])
                        if split:
                            h = int(s0 >= khalf)
                            t0 = s0 - h * khalf
                            nc.sync.dma_start(
                                out=hp_bounce[h][:].rearrange(
                                    "(t p) o -> p t o",
                                    p=P)[:, t0:t0 + s1 - s0, :],
                                in_=strip[:, :s1 - s0, :])
                            if s1 == khalf and "ag" not in skip:
                                nc.gpsimd.collective_compute(
                                    "AllGather", mybir.AluOpType.bypass,
                                    ins=[hp_bounce[0].opt()],
                                    outs=[hp_full[0].opt()],
                                    replica_groups=replica)
                        else:
                            nc.sync.dma_start(
                                out=hp_bounce[:].rearrange(
                                    "(t p) o -> p t o",
                                    p=P)[:, s0:s1, :OUT],
                                in_=strip[:, :s1 - s0, :])

                if "ag" not in skip:
                    if split:
                        nc.gpsimd.collective_compute(
                            "AllGather", mybir.AluOpType.bypass,
                            ins=[hp_bounce[1].opt()],
                            outs=[hp_full[1].opt()],
                            replica_groups=replica)
                    else:
                        nc.gpsimd.collective_compute(
                            "AllGather", mybir.AluOpType.bypass,
                            ins=[hp_bounce.opt()], outs=[hp_full.opt()],
                            replica_groups=replica)

                # ------------- P2: gather+aggregate layer 1 -> hs ------
                with tc.tile_pool(name="g1", bufs=3) as gpool, \
                     tc.tile_pool(name="s1", bufs=4) as spool, \
                     tc.tile_pool(name="h1", bufs=4) as hpool, \
                     tc.tile_pool(name="hss", bufs=2) as hsspool:

                    hs_strips = {}
                    hs_counts = {}

                    def sink1(kk, ps):
                        g = kk // SG
                        j = kk % SG
                        sgsz = min(SG, nsb - g * SG)
                        if g not in hs_strips:
                            hs_strips[g] = hsspool.tile(
                                [P, SG, P], BF16, tag="hss", name=f"hss_{g}")
                            hs_counts[g] = 0
                        t1 = hpool.tile([P, OUT], F32, tag="t1", name="t1")
                        nc.vector.scalar_tensor_tensor(
                            out=t1[:], in0=ps, scalar=nd_t[:, kk:kk + 1],
                            in1=b1_t[:], op0=mybir.AluOpType.mult,
                            op1=mybir.AluOpType.add)
                        nc.scalar.activation(
                            hs_strips[g][:, j, :], t1[:],
                            mybir.ActivationFunctionType.Relu,
                            scale=ns_t[:, kk:kk + 1])
                        hs_counts[g] += 1
                        if hs_counts[g] == sgsz:
                            k0 = g * SG
                            if split:
                                na = min(max(khalf - k0, 0), sgsz)
                                if na:
                                    nc.sync.dma_start(
                                        out=hs_bounce[0][:].rearrange(
                                            "(t p) o -> p t o",
                                            p=P)[:, k0:k0 + na, :],
                                        in_=hs_strips[g][:, :na, :])
                                if na < sgsz:
                                    t0 = max(k0 - khalf, 0)
                                    nc.sync.dma_start(
                                        out=hs_bounce[1][:].rearrange(
                                            "(t p) o -> p t o",
                                            p=P)[:, t0:t0 + sgsz - na, :],
                                        in_=hs_strips[g][:, na:sgsz, :])
                                if (k0 < khalf <= k0 + sgsz
                                        and "ag" not in skip
                                        and not sink1.ag_done):
                                    sink1.ag_done = True
                                    nc.gpsimd.collective_compute(
                                        "AllGather",
                                        mybir.AluOpType.bypass,
                                        ins=[hs_bounce[0].opt()],
                                        outs=[hs_full[0].opt()],
                                        replica_groups=replica)
                            else:
                                nc.sync.dma_start(
                                    out=hs_bounce[:].rearrange(
                                        "(t p) o -> p t o",
                                        p=P)[:, k0:k0 + sgsz, :OUT],
                                    in_=hs_strips[g][:, :sgsz, :])

                    sink1.ag_done = False
                    if "gather" not in skip:
                        if split:
                            tabs = [hp_full[0][0:brows, :],
                                    hp_full[0][brows:2 * brows, :],
                                    hp_full[1][0:brows, :],
                                    hp_full[1][brows:2 * brows, :]]
                        else:
                            tabs = [hp_full[bb * brows:(bb + 1) * brows, :]
                                    for bb in range(NBUCK)]
                        gather_pass(tabs, gpool, spool, False, sink1)

                if "ag" not in skip:
                    if split:
                        nc.gpsimd.collective_compute(
                            "AllGather", mybir.AluOpType.bypass,
                            ins=[hs_bounce[1].opt()],
                            outs=[hs_full[1].opt()],
                            replica_groups=replica)
                    else:
                        nc.gpsimd.collective_compute(
                            "AllGather", mybir.AluOpType.bypass,
                            ins=[hs_bounce.opt()], outs=[hs_full.opt()],
                            replica_groups=replica)

                # ------------- P4: gather+aggregate layers 2/3 ----------
                with tc.tile_pool(name="g2", bufs=3) as gpool2, \
                     tc.tile_pool(name="s2", bufs=4) as spool2, \
                     tc.tile_pool(name="e2", bufs=6) as epool, \
                     tc.tile_pool(name="noisep", bufs=1) as npool, \
                     tc.tile_pool(name="outs", bufs=2) as outpool:

                    noise_t = npool.tile([P, nsb, OUT], F32, tag="noise",
                                         name="noise")
                    nc.sync.dma_start(out=noise_t[:],
                                      in_=noise_d[:].rearrange(
                                          "p (k o) -> p k o", k=nsb))

                    out_strips = {}
                    out_counts = {}

                    def sink2(kk, psT):
                        g = kk // SG
                        j = kk % SG
                        sgsz = min(SG, nsb - g * SG)
                        if g not in out_strips:
                            out_strips[g] = outpool.tile(
                                [P, SG, OUT], F32, tag="outs",
                                name=f"os_{g}")
                            out_counts[g] = 0
                        aggT = epool.tile([P, P], BF16, tag="aggT",
                                          name="aggT")
                        nc.scalar.activation(
                            aggT[:], psT,
                            mybir.ActivationFunctionType.Copy)
                        ps2 = psA.tile([P, F2], F32, tag="acc",
                                       name="mm2")
                        nc.tensor.matmul(ps2[:], lhsT=aggT[:], rhs=W23_t[:],
                                         start=True, stop=True)
                        tmu = epool.tile([P, OUT], F32, tag="tmu", name="tmu")
                        nc.vector.scalar_tensor_tensor(
                            out=tmu[:], in0=ps2[:, 0:OUT],
                            scalar=nd_t[:, kk:kk + 1], in1=bmu_t[:],
                            op0=mybir.AluOpType.mult,
                            op1=mybir.AluOpType.add)
                        tls = epool.tile([P, OUT], F32, tag="tls", name="tls")
                        nc.vector.scalar_tensor_tensor(
                            out=tls[:], in0=ps2[:, OUT:F2],
                            scalar=nd_t[:, kk:kk + 1], in1=bls_t[:],
                            op0=mybir.AluOpType.mult,
                            op1=mybir.AluOpType.add)
                        sig = epool.tile([P, OUT], F32, tag="sig", name="sig")
                        nc.scalar.activation(
                            sig[:], tls[:],
                            mybir.ActivationFunctionType.Exp)
                        nc.vector.tensor_tensor(out=sig[:], in0=sig[:],
                                                in1=noise_t[:, kk, :],
                                                op=mybir.AluOpType.mult)
                        nc.vector.tensor_tensor(out=out_strips[g][:, j, :],
                                                in0=tmu[:], in1=sig[:],
                                                op=mybir.AluOpType.add)
                        out_counts[g] += 1
                        if out_counts[g] == sgsz:
                            k0 = g * SG
                            nc.sync.dma_start(
                                out=y_d[:].rearrange("(t p) o -> p t o",
                                                     p=P)[:, k0:k0 + sgsz, :],
                                in_=out_strips[g][:, :sgsz, :])

                    if "gather" not in skip:
                        if split:
                            tabs2 = [hs_full[0][0:brows, :],
                                     hs_full[0][brows:2 * brows, :],
                                     hs_full[1][0:brows, :],
                                     hs_full[1][brows:2 * brows, :]]
                        else:
                            tabs2 = [hs_full[bb * brows:(bb + 1) * brows, :]
                                     for bb in range(NBUCK)]
                        gather_pass(tabs2, gpool2, spool2, True, sink2)

            for _rep in range(repeat):
                one_iter(hp_fulls[_rep], hs_fulls[_rep])

    nc.compile()
    return nc, in_maps, N


_CACHE = {}


def _run(feat, edges, W1, b1, W_mu, b_mu, W_ls, b_ls, noise, trace=False):
    import hashlib
    h = hashlib.sha1()
    for a in (edges, feat, W1, b1, W_mu, b_mu, W_ls, b_ls, noise):
        h.update(np.ascontiguousarray(a).tobytes())
    key = h.hexdigest()
    if key in _CACHE:
        nc, in_maps, N = _CACHE[key]
    else:
        nc, in_maps, N = _build(feat, edges, W1, b1, W_mu, b_mu, W_ls, b_ls,
                                noise)
        _CACHE[key] = (nc, in_maps, N)
    res = run_bass_kernel_spmd(nc, in_maps, core_ids=list(range(NC)),
                               trace=trace)
    out = np.concatenate([res.results[c]["y"] for c in range(NC)], axis=0)
    return out[:N], res


def kernel(feat, edges, W1, b1, W_mu, b_mu, W_ls, b_ls, noise):
    out, _ = _run(np.asarray(feat), np.asarray(edges), np.asarray(W1),
                  np.asarray(b1), np.asarray(W_mu), np.asarray(b_mu),
                  np.asarray(W_ls), np.asarray(b_ls), np.asarray(noise))
    return out
